# revision 1
# baseline (speedup 1.0000x reference)
"""Trainium2 Bass kernel for DifferentiableRGBtoVel (soft-nearest-neighbor
colormap inversion).

velocity(p) = sum_k v_k e^{-100 d_k(p)} / sum_k e^{-100 d_k(p)},
d_k(p) = |p - c_k|^2.

Softmax stabilizer: the linear surrogate B_p = 100*sum_c(p_c) - 37.5 of
100|p|^2 (minimax linear fit of x^2 on [0,1]) keeps every exponent inside
fp32 range; the shift cancels exactly in the num/den ratio.

All matmuls are genuine fp32 (float32r truncates operands to 11 mantissa
bits, and mixing f32r with fp32 matmuls corrupts the fp32 ones on this
silicon -- measured, deterministic, schedule-dependent).

Per-core pipeline in [k, pix] layout (partition = colormap index), tiles of
512 pixels ([128, 1024] PSUM = [A|B]):
  scores: one K=4 fp32 matmul per 128-color half; halves run concurrently in
          PE row groups 0/32 (image rows duplicated to partitions 32-35).
          psum[k,px] = sum_c (c_kc-0.5) p_c + (37.5-100|c_k|^2)/200 * 1
  exp:    ONE ACT instruction per tile (FD=1024), func=Exp, scale=200.
  num/den: fp32 [128,2] matmuls (cols = [1, v_k]); each tile's chain rotates
          over the 4 PE column groups so up to 4 chains run concurrently.
          Output lands in a corner of the already-consumed score PSUM tile.
  divide: DVE copy -> SBUF, partition-compacting DMA into dense [128,512]
          accumulators (den rows 0-63, num rows 64-127), DVE reciprocal +
          multiply per 64-tile group, one output DMA per group.
"""

import numpy as np

import concourse.bass as bass
import concourse.mybir as mybir
import concourse.tile as tile_mod
from concourse.tile import TileContext
from concourse.vector_clock import ScopedClock, VectorClock
from concourse.bass_utils import run_bass_kernel_spmd

# ---------------------------------------------------------------- constants
N_CORES = 8
NB, C, H, W = 4, 3, 512, 512
K = 256
KH = 128
PIX_PER_CORE = NB * H * W // N_CORES   # 131072
TILE_PIX = 512                 # pixels per tile
GROUP = 64                     # tiles per division group
IMG_BATCH = 8                  # tiles per image DMA

_FP32 = mybir.dt.float32


# ------------------------------------------------- walrus sync-wait limits
# This walrus build rejects instructions carrying more than one sem wait
# ("Too many sync wait commands"); split extras onto same-engine NoOps.
def _split_drain_and_barrier(self, tick_clock, wait_clock):
    nc = self.nc
    vec = list(tick_clock.global_clock)
    for i, v in enumerate(vec):
        if v > 0:
            w = [0] * len(vec)
            w[i] = v
            inst = nc.sync.nop(nofuse=True, hint="split_drain_wait")
            wait_clock.add_sem_waits(inst.ins, ScopedClock({None: VectorClock(w)}))
    nc.sync.drain()
    nc.all_engine_barrier()
    assert self.sems is not None
    popped = nc._tile_sem_poison_stack.pop()
    assert popped is self._sem_poison
    nc.clear_and_free_semaphores(list(self.sems.allocated().values()))
    nc.all_engine_barrier()


tile_mod.TileContext._drain_and_barrier = _split_drain_and_barrier

MAX_WAITS = 1


def _split_excess_waits(nc, maxw=MAX_WAITS):
    for f in nc.m.functions:
        for bb in f.blocks:
            out = []
            for inst in bb.instructions:
                si = inst.sync_info
                if si is not None and len(si.on_wait) > maxw:
                    waits = list(si.on_wait)
                    excess, keep = waits[:-maxw], waits[-maxw:]
                    for i in range(0, len(excess), maxw):
                        nop = mybir.InstNoOp(
                            name=nc.get_next_instruction_name(),
                            sync_info=mybir.SyncInfo(
                                on_wait=excess[i:i + maxw], on_update=[]),
                            bass_nofuse=True,
                            engine=inst.engine,
                        )
                        out.append(nop)
                    inst.sync_info = mybir.SyncInfo(
                        on_wait=keep, on_update=list(si.on_update))
                out.append(inst)
            bb.instructions = out


# ------------------------------------------------------------- bass builder
def build_kernel(pix_per_core: int = PIX_PER_CORE):
    n_tiles = pix_per_core // TILE_PIX
    n_groups = (n_tiles + GROUP - 1) // GROUP

    nc = bass.Bass(trn_type="TRN2", name="rgb2vel")
    imgD = nc.dram_tensor("img", [4, pix_per_core], _FP32, kind="ExternalInput")
    cmD = nc.dram_tensor("cmt", [4, K], _FP32, kind="ExternalInput")
    vmD = nc.dram_tensor("vmat", [KH, 4], _FP32, kind="ExternalInput")
    velD = nc.dram_tensor("vel", [pix_per_core // 512, 512], _FP32,
                          kind="ExternalOutput")

    ExpF = mybir.ActivationFunctionType.Exp

    with TileContext(nc) as tc:
        with (
            tc.tile_pool(name="const", bufs=1) as cpool,
            tc.tile_pool(name="img", bufs=3) as ipool,
            tc.tile_pool(name="exp", bufs=6) as epool,
            tc.tile_pool(name="stg", bufs=8) as stgpool,
            tc.tile_pool(name="acc", bufs=2) as accpool,
            tc.tile_pool(name="divp", bufs=2) as dpool,
            tc.tile_pool(name="score", bufs=4, space="PSUM") as spool,
        ):
            # persistent constants: cm rows 0-3 = half A, rows 32-35 = half B
            cm = cpool.tile([36, KH], _FP32, tag="cm")
            nc.sync.dma_start(cm[0:4, :], cmD[:, 0:KH])
            nc.sync.dma_start(cm[32:36, :], cmD[:, KH:K])
            vm = cpool.tile([KH, 4], _FP32, tag="vm")
            nc.sync.dma_start(vm[:], vmD[:])

            state = {"img": None, "pending": []}

            def emit_tail(dnv):
                # V chain + evacuation for the oldest pending tile
                t, j, ps, ex = state["pending"].pop(0)
                m = t % 4
                tp = (0, 32 * m) if m else None
                out = ps[32 * m:32 * m + 2, 0:TILE_PIX]
                nc.tensor.matmul(out, lhsT=vm[:, 0:2], rhs=ex[:, 0:TILE_PIX],
                                 start=True, stop=False, tile_position=tp)
                nc.tensor.matmul(out, lhsT=vm[:, 2:4],
                                 rhs=ex[:, TILE_PIX:2 * TILE_PIX],
                                 start=False, stop=True, tile_position=tp)
                stg = stgpool.tile([98, TILE_PIX], _FP32, tag="stg")
                nc.vector.tensor_copy(stg[32 * m:32 * m + 2, :], out)
                nc.sync.dma_start(dnv[:, j, :], stg[32 * m:32 * m + 2, :])

            def do_tile(t, dnv, j):
                if t % IMG_BATCH == 0:
                    imgt = ipool.tile([36, IMG_BATCH * TILE_PIX], _FP32,
                                      tag="img")
                    sl = slice(t * TILE_PIX, (t + IMG_BATCH) * TILE_PIX)
                    nc.sync.dma_start(imgt[0:4, :], imgD[:, sl])
                    nc.sync.dma_start(imgt[32:36, :], imgD[:, sl])
                    state["img"] = imgt
                img = state["img"]
                ioff = (t % IMG_BATCH) * TILE_PIX
                isl = slice(ioff, ioff + TILE_PIX)

                # scores [A|B], halves concurrent in row groups 0/32
                ps = spool.tile([128, 2 * TILE_PIX], _FP32, tag="score")
                nc.tensor.matmul(ps[:, 0:TILE_PIX], lhsT=cm[0:4, :],
                                 rhs=img[0:4, isl], start=True, stop=True)
                nc.tensor.matmul(ps[:, TILE_PIX:2 * TILE_PIX],
                                 lhsT=cm[32:36, :], rhs=img[32:36, isl],
                                 start=True, stop=True)

                ex = epool.tile([128, 2 * TILE_PIX], _FP32, tag="exp")
                nc.scalar.activation(ex[:], ps[:], ExpF, bias=0.0, scale=200.0)
                state["pending"].append((t, j, ps, ex))
                if len(state["pending"]) >= 3:
                    # emit two V chains back-to-back: consecutive tiles use
                    # different PE column groups, so adjacent chains overlap
                    emit_tail(dnv)
                    emit_tail(dnv)

            def do_group(g, gtiles):
                dn = accpool.tile([128, 512], _FP32, tag="dn")
                dnv = dn.rearrange("(a p) w -> a p w", a=2)
                for j in range(gtiles):
                    do_tile(g * GROUP + j, dnv, j)
                while state["pending"]:
                    emit_tail(dnv)
                rows = gtiles
                nsh = dpool.tile([64, 512], _FP32, tag="nsh")
                nc.sync.dma_start(nsh[0:rows, :], dn[64:64 + rows, :])
                rcp = dpool.tile([64, 512], _FP32, tag="rcp")
                nc.vector.reciprocal(rcp[0:rows, :], dn[0:rows, :])
                vel = dpool.tile([64, 512], _FP32, tag="vel")
                nc.vector.tensor_tensor(
                    vel[0:rows, :], nsh[0:rows, :], rcp[0:rows, :],
                    mybir.AluOpType.mult)
                nc.sync.dma_start(velD[g * GROUP:g * GROUP + rows, :],
                                  vel[0:rows, :])

            for g in range(n_groups):
                do_group(g, min(GROUP, n_tiles - g * GROUP))

    _split_excess_waits(nc)
    return nc


# ----------------------------------------------------------- host wrapper
_CACHE = {}


def _get_nc(pix_per_core):
    if pix_per_core not in _CACHE:
        _CACHE[pix_per_core] = build_kernel(pix_per_core)
    return _CACHE[pix_per_core]


def _prep_consts(cmap, v_i):
    cmap = np.asarray(cmap, np.float32)
    v_i = np.asarray(v_i, np.float32)
    c2 = np.sum(cmap * cmap, axis=1, dtype=np.float32)
    cmt = np.empty((4, K), np.float32)
    cmt[0:3, :] = (cmap.T - np.float32(0.5))
    cmt[3, :] = (np.float32(37.5) - np.float32(100.0) * c2) / np.float32(200.0)
    vmat = np.empty((KH, 4), np.float32)
    vmat[:, 0] = 1.0
    vmat[:, 1] = v_i[0:KH]
    vmat[:, 2] = 1.0
    vmat[:, 3] = v_i[KH:K]
    return cmt, vmat


def _prep_image_slab(slab):
    """slab: [3, n] float32 -> [4, n] rows [r, g, b, ones]."""
    n = slab.shape[1]
    img = np.empty((4, n), np.float32)
    img[0:3] = slab
    img[3] = 1.0
    return img


def _kernel_impl(image, cmap, v_i, _trace=False):
    image = np.ascontiguousarray(np.asarray(image, np.float32))
    cmt, vmat = _prep_consts(cmap, v_i)

    rows_per_core = NB * H // N_CORES          # 256 rows of H per core
    in_maps = []
    for i in range(N_CORES):
        n = (i * rows_per_core) // H
        h0 = (i * rows_per_core) % H
        slab = image[n, :, h0:h0 + rows_per_core, :].reshape(3, -1)
        in_maps.append({"img": _prep_image_slab(slab), "cmt": cmt,
                        "vmat": vmat})

    nc = _get_nc(PIX_PER_CORE)
    res = run_bass_kernel_spmd(nc, in_maps, core_ids=list(range(N_CORES)),
                               trace=_trace)
    out = np.empty((NB, H, W), np.float32)
    for i in range(N_CORES):
        n = (i * rows_per_core) // H
        h0 = (i * rows_per_core) % H
        out[n, h0:h0 + rows_per_core, :] = \
            res.results[i]["vel"].reshape(rows_per_core, W)
    return out, res


def kernel(image, cmap, v_i):
    out, _ = _kernel_impl(image, cmap, v_i)
    return out



# revision 2
# speedup vs baseline: 1.2946x; 1.2946x over previous
"""Trainium2 Bass kernel for DifferentiableRGBtoVel (soft-nearest-neighbor
colormap inversion).

velocity(p) = sum_k v_k e^{-100 d_k(p)} / sum_k e^{-100 d_k(p)},
d_k(p) = |p - c_k|^2.

Softmax stabilizer: the linear surrogate B_p = 100*sum_c(p_c) - 37.5 of
100|p|^2 (minimax linear fit of x^2 on [0,1]) keeps every exponent inside
fp32 range; the shift cancels exactly in the num/den ratio.

All matmuls are bf16 (1 cycle/col on PE vs 4 for fp32).  Score precision is
recovered with a split-bf16 contraction: p = ph + pl, c' = chi + clo
(each split exact to ~2^-17), and

  s/200 = chi.ph + chi.pl + clo.ph + qhi + qlo      (drops clo.pl ~ 1e-6)

as a single K=11 matmul with rhs rows [ph(3), pl(3), ph(3), 1, 1] and
weight rows [chi(3), chi(3), clo(3), qhi, qlo].  End-to-end error vs the
fp32 reference is ~4e-3 (dominated by bf16 rounding of e^s and v_k), vs a
2e-2 gate.

Per-core pipeline in [k, pix] layout (partition = colormap index), tiles of
512 pixels ([128, 1024] PSUM = [A|B]):
  scores: one K=11 bf16 matmul per 128-color half; halves run concurrently
          in PE row groups 0/32 (image rows duplicated to partitions 32-42).
  exp:    ONE ACT instruction per tile (FD=1024), func=Exp, scale=200,
          output bf16 straight to SBUF.
  num/den: bf16 [128,2] matmuls (cols = [1, v_k]); each tile's chain rotates
          over the 4 PE column groups so up to 4 chains run concurrently.
          Output lands in a corner of the already-consumed score PSUM tile.
  divide: DVE copy -> SBUF, partition-compacting DMA into dense [128,512]
          accumulators (den rows 0-63, num rows 64-127), DVE reciprocal +
          multiply per 64-tile group, one output DMA per group.
"""

import numpy as np
import ml_dtypes

import concourse.bass as bass
import concourse.mybir as mybir
import concourse.tile as tile_mod
from concourse.tile import TileContext
from concourse.vector_clock import ScopedClock, VectorClock
from concourse.bass_utils import run_bass_kernel_spmd

# ---------------------------------------------------------------- constants
N_CORES = 8
NB, C, H, W = 4, 3, 512, 512
K = 256
KH = 128
NR = 11                        # contraction rows: ph(3) pl(3) ph(3) 1 1
PIX_PER_CORE = NB * H * W // N_CORES   # 131072
TILE_PIX = 512                 # pixels per tile
GROUP = 64                     # tiles per division group
IMG_BATCH = 8                  # tiles per image DMA

_FP32 = mybir.dt.float32
_BF16 = mybir.dt.bfloat16
_NPBF = ml_dtypes.bfloat16


# ------------------------------------------------- walrus sync-wait limits
# This walrus build rejects instructions carrying more than one sem wait
# ("Too many sync wait commands"); split extras onto same-engine NoOps.
def _split_drain_and_barrier(self, tick_clock, wait_clock):
    nc = self.nc
    vec = list(tick_clock.global_clock)
    for i, v in enumerate(vec):
        if v > 0:
            w = [0] * len(vec)
            w[i] = v
            inst = nc.sync.nop(nofuse=True, hint="split_drain_wait")
            wait_clock.add_sem_waits(inst.ins, ScopedClock({None: VectorClock(w)}))
    nc.sync.drain()
    nc.all_engine_barrier()
    assert self.sems is not None
    popped = nc._tile_sem_poison_stack.pop()
    assert popped is self._sem_poison
    nc.clear_and_free_semaphores(list(self.sems.allocated().values()))
    nc.all_engine_barrier()


tile_mod.TileContext._drain_and_barrier = _split_drain_and_barrier

MAX_WAITS = 1


def _split_excess_waits(nc, maxw=MAX_WAITS):
    for f in nc.m.functions:
        for bb in f.blocks:
            out = []
            for inst in bb.instructions:
                si = inst.sync_info
                if si is not None and len(si.on_wait) > maxw:
                    waits = list(si.on_wait)
                    excess, keep = waits[:-maxw], waits[-maxw:]
                    for i in range(0, len(excess), maxw):
                        nop = mybir.InstNoOp(
                            name=nc.get_next_instruction_name(),
                            sync_info=mybir.SyncInfo(
                                on_wait=excess[i:i + maxw], on_update=[]),
                            bass_nofuse=True,
                            engine=inst.engine,
                        )
                        out.append(nop)
                    inst.sync_info = mybir.SyncInfo(
                        on_wait=keep, on_update=list(si.on_update))
                out.append(inst)
            bb.instructions = out


# ------------------------------------------------------------- bass builder
def build_kernel(pix_per_core: int = PIX_PER_CORE):
    n_tiles = pix_per_core // TILE_PIX
    n_groups = (n_tiles + GROUP - 1) // GROUP

    nc = bass.Bass(trn_type="TRN2", name="rgb2vel")
    imgD = nc.dram_tensor("img", [NR, pix_per_core], _BF16, kind="ExternalInput")
    cmD = nc.dram_tensor("cmt", [NR, K], _BF16, kind="ExternalInput")
    vmD = nc.dram_tensor("vmat", [KH, 4], _BF16, kind="ExternalInput")
    velD = nc.dram_tensor("vel", [pix_per_core // 512, 512], _FP32,
                          kind="ExternalOutput")

    ExpF = mybir.ActivationFunctionType.Exp

    with TileContext(nc) as tc:
        with (
            tc.tile_pool(name="const", bufs=1) as cpool,
            tc.tile_pool(name="img", bufs=3) as ipool,
            tc.tile_pool(name="exp", bufs=6) as epool,
            tc.tile_pool(name="stg", bufs=8) as stgpool,
            tc.tile_pool(name="acc", bufs=2) as accpool,
            tc.tile_pool(name="divp", bufs=2) as dpool,
            tc.tile_pool(name="score", bufs=4, space="PSUM") as spool,
        ):
            # persistent constants: cm rows 0-10 = half A, rows 32-42 = half B
            cm = cpool.tile([32 + NR, KH], _BF16, tag="cm")
            nc.sync.dma_start(cm[0:NR, :], cmD[:, 0:KH])
            nc.sync.dma_start(cm[32:32 + NR, :], cmD[:, KH:K])
            vm = cpool.tile([KH, 4], _BF16, tag="vm")
            nc.sync.dma_start(vm[:], vmD[:])

            state = {"img": None, "pending": []}

            def emit_tail(dnv):
                # V chain + evacuation for the oldest pending tile
                t, j, ps, ex = state["pending"].pop(0)
                m = t % 4
                tp = (0, 32 * m) if m else None
                out = ps[32 * m:32 * m + 2, 0:TILE_PIX]
                nc.tensor.matmul(out, lhsT=vm[:, 0:2], rhs=ex[:, 0:TILE_PIX],
                                 start=True, stop=False, tile_position=tp)
                nc.tensor.matmul(out, lhsT=vm[:, 2:4],
                                 rhs=ex[:, TILE_PIX:2 * TILE_PIX],
                                 start=False, stop=True, tile_position=tp)
                stg = stgpool.tile([98, TILE_PIX], _FP32, tag="stg")
                nc.vector.tensor_copy(stg[32 * m:32 * m + 2, :], out)
                nc.sync.dma_start(dnv[:, j, :], stg[32 * m:32 * m + 2, :])

            def do_tile(t, dnv, j):
                if t % IMG_BATCH == 0:
                    imgt = ipool.tile([32 + NR, IMG_BATCH * TILE_PIX], _BF16,
                                      tag="img")
                    sl = slice(t * TILE_PIX, (t + IMG_BATCH) * TILE_PIX)
                    nc.sync.dma_start(imgt[0:NR, :], imgD[:, sl])
                    nc.sync.dma_start(imgt[32:32 + NR, :], imgD[:, sl])
                    state["img"] = imgt
                img = state["img"]
                ioff = (t % IMG_BATCH) * TILE_PIX
                isl = slice(ioff, ioff + TILE_PIX)

                # scores [A|B], halves concurrent in row groups 0/32
                ps = spool.tile([128, 2 * TILE_PIX], _FP32, tag="score")
                nc.tensor.matmul(ps[:, 0:TILE_PIX], lhsT=cm[0:NR, :],
                                 rhs=img[0:NR, isl], start=True, stop=True)
                nc.tensor.matmul(ps[:, TILE_PIX:2 * TILE_PIX],
                                 lhsT=cm[32:32 + NR, :],
                                 rhs=img[32:32 + NR, isl],
                                 start=True, stop=True)

                ex = epool.tile([128, 2 * TILE_PIX], _BF16, tag="exp")
                nc.scalar.activation(ex[:], ps[:], ExpF, bias=0.0, scale=200.0)
                state["pending"].append((t, j, ps, ex))
                if len(state["pending"]) >= 3:
                    # emit two V chains back-to-back: consecutive tiles use
                    # different PE column groups, so adjacent chains overlap
                    emit_tail(dnv)
                    emit_tail(dnv)

            def do_group(g, gtiles):
                dn = accpool.tile([128, 512], _FP32, tag="dn")
                dnv = dn.rearrange("(a p) w -> a p w", a=2)
                for j in range(gtiles):
                    do_tile(g * GROUP + j, dnv, j)
                while state["pending"]:
                    emit_tail(dnv)
                rows = gtiles
                nsh = dpool.tile([64, 512], _FP32, tag="nsh")
                nc.sync.dma_start(nsh[0:rows, :], dn[64:64 + rows, :])
                rcp = dpool.tile([64, 512], _FP32, tag="rcp")
                nc.vector.reciprocal(rcp[0:rows, :], dn[0:rows, :])
                vel = dpool.tile([64, 512], _FP32, tag="vel")
                nc.vector.tensor_tensor(
                    vel[0:rows, :], nsh[0:rows, :], rcp[0:rows, :],
                    mybir.AluOpType.mult)
                nc.sync.dma_start(velD[g * GROUP:g * GROUP + rows, :],
                                  vel[0:rows, :])

            for g in range(n_groups):
                do_group(g, min(GROUP, n_tiles - g * GROUP))

    _split_excess_waits(nc)
    return nc


# ----------------------------------------------------------- host wrapper
_CACHE = {}


def _get_nc(pix_per_core):
    if pix_per_core not in _CACHE:
        _CACHE[pix_per_core] = build_kernel(pix_per_core)
    return _CACHE[pix_per_core]


def _prep_consts(cmap, v_i):
    cmap = np.asarray(cmap, np.float32)
    v_i = np.asarray(v_i, np.float32)
    cc = cmap.T - np.float32(0.5)                      # [3, 256]
    chi = cc.astype(_NPBF)
    clo = (cc - chi.astype(np.float32)).astype(_NPBF)
    c2 = np.sum(cmap * cmap, axis=1, dtype=np.float32)
    q = (np.float32(37.5) - np.float32(100.0) * c2) / np.float32(200.0)
    qhi = q.astype(_NPBF)
    qlo = (q - qhi.astype(np.float32)).astype(_NPBF)
    cmt = np.empty((NR, K), _NPBF)
    cmt[0:3] = chi
    cmt[3:6] = chi
    cmt[6:9] = clo
    cmt[9] = qhi
    cmt[10] = qlo
    vmat = np.empty((KH, 4), _NPBF)
    vmat[:, 0] = np.float32(1.0)
    vmat[:, 1] = v_i[0:KH].astype(_NPBF)
    vmat[:, 2] = np.float32(1.0)
    vmat[:, 3] = v_i[KH:K].astype(_NPBF)
    return cmt, vmat


def _prep_image_slab(slab):
    """slab: [3, n] float32 -> [NR, n] bf16 rows [ph, pl, ph, 1, 1]."""
    n = slab.shape[1]
    ph = slab.astype(_NPBF)
    pl = (slab - ph.astype(np.float32)).astype(_NPBF)
    img = np.empty((NR, n), _NPBF)
    img[0:3] = ph
    img[3:6] = pl
    img[6:9] = ph
    img[9] = np.float32(1.0)
    img[10] = np.float32(1.0)
    return img


def _kernel_impl(image, cmap, v_i, _trace=False):
    image = np.ascontiguousarray(np.asarray(image, np.float32))
    cmt, vmat = _prep_consts(cmap, v_i)

    rows_per_core = NB * H // N_CORES          # 256 rows of H per core
    in_maps = []
    for i in range(N_CORES):
        n = (i * rows_per_core) // H
        h0 = (i * rows_per_core) % H
        slab = image[n, :, h0:h0 + rows_per_core, :].reshape(3, -1)
        in_maps.append({"img": _prep_image_slab(slab), "cmt": cmt,
                        "vmat": vmat})

    nc = _get_nc(PIX_PER_CORE)
    res = run_bass_kernel_spmd(nc, in_maps, core_ids=list(range(N_CORES)),
                               trace=_trace)
    out = np.empty((NB, H, W), np.float32)
    for i in range(N_CORES):
        n = (i * rows_per_core) // H
        h0 = (i * rows_per_core) % H
        out[n, h0:h0 + rows_per_core, :] = \
            res.results[i]["vel"].reshape(rows_per_core, W)
    return out, res


def kernel(image, cmap, v_i):
    out, _ = _kernel_impl(image, cmap, v_i)
    return out
